# revision 3
# baseline (speedup 1.0000x reference)
"""Trainium2 Bass kernel for nn_TinyLPR_64845416235418.

The reference network is structurally degenerate: every `sign` that follows a
`relu` is identically +1 (ste_sign maps 0 -> +1 and relu outputs are >= 0).
In particular the final logits are

    logits[b, t, o] = sum_f sign(rb2_out)[b,t,f] * sign(wo[f, o])
                    = sum_f sign(wo[f, o])            (sign(relu(...)) == +1)

independent of x and of every parameter except `wo`.  The exact output of the
reference for ANY input is therefore

    out[b, t, :] = softmax(colsum(sign(wo)))          broadcast over (b, t)

(verified bit-for-bit against the jax reference on the neuron stack: the
reference output is constant across (b, t) with deviation exactly 0.0, and
matches softmax(colsum(sign(wo))) to 3e-20).

Sharding: pure data parallel over the batch dim — each of the 8 cores
computes its 16-image shard of the output.  Every core receives the (tiny)
replicated `wo`, reshaped host-side to [32, 340] so the HBM->SBUF DMA is 32
fat descriptors instead of 128 thin ones.  On device (per core):

    g   = 2 * (wo >= 0)                  in {0,2}, bf16     (DVE, one dual-op)
    r1  = sum of g over the 4 f-values
          packed per partition           [32, 85], exact    (DVE strided reduce)
    2C  = ones[32,32]^T @ r1             -> [32, 85] psum, broadcast over
                                            partitions      (PE, bf16 exact)
    nb  = -rowmax(2C)                                       (DVE, negated max)
    e   = exp(2C + nb)                                      (ACT; exp table
                                            pre-warmed during the DMA)
    p   = e / rowsum(e)                  final multiply fused with 4-fold
                                         free-dim replication -> [32, 340]
    out[32, 340] == row-major (16, 8, 85) shard             (32-descriptor DMA)

All count arithmetic is exact (integers in bf16/fp32), so the only rounding
is in exp/sum/div — measured bit-identical to the jax fp32 softmax on HW.
"""

import numpy as np

import concourse.bass as bass
import concourse.mybir as mybir
from concourse.bass_utils import run_bass_kernel_spmd

N_CORES = 8
B_FULL = 128   # full batch
T = 8          # time steps
O = 85         # classes
F = 128        # wo rows
B_SHARD = B_FULL // N_CORES   # 16 images per core

P = 32                # SBUF partitions used for the wo layout
KPP = F // P          # f-values per partition (4)
FREE = KPP * O        # 340 free elements per partition

_F32 = mybir.dt.float32
_BF16 = mybir.dt.bfloat16


def build_nc() -> bass.Bass:
    nc = bass.Bass(enable_partition_id=False)
    wo_ext = nc.declare_dram_parameter("wo", [P, FREE], _F32, isOutput=False)
    out_ext = nc.declare_dram_parameter("out", [P, FREE], _F32, isOutput=True)

    with (
        nc.sbuf_tensor([P, FREE], _F32) as a_sb,    # wo
        nc.sbuf_tensor([P, FREE], _BF16) as g_sb,   # 2*(wo>=0)
        nc.sbuf_tensor([P, O], _BF16) as r1_sb,     # per-partition partial counts
        nc.sbuf_tensor([P, P], _BF16) as ones_sb,   # matmul lhsT of ones
        nc.sbuf_tensor([P, 1], _F32) as nb_sb,      # -rowmax(2C)
        nc.sbuf_tensor([P, O], _F32) as e_sb,       # exp(2C - max)
        nc.sbuf_tensor([P, 1], _F32) as s_sb,       # rowsum(e)
        nc.sbuf_tensor([P, 1], _F32) as r_sb,       # 1/rowsum(e)
        nc.sbuf_tensor([P, FREE], _F32) as p_sb,    # output tile
        nc.sbuf_tensor([1, 1], _F32) as scr_sb,     # exp-table prewarm scratch
        nc.psum_tensor([P, O], _F32) as c_ps,       # 2C counts
        nc.semaphore("dma_sem") as dma_sem,
        nc.semaphore("sem") as sem,
        nc.Block() as block,
    ):

        @block.sync
        def _(sync):
            sync.dma_start(out=a_sb[:], in_=wo_ext[:]).then_inc(dma_sem, 16)
            sync.wait_ge(sem, 12)
            sync.dma_start(out=out_ext[:], in_=p_sb[:]).then_inc(dma_sem, 16)
            sync.wait_ge(dma_sem, 32)

        @block.vector
        def _(vector):
            vector.memset(ones_sb[:], 1.0).then_inc(sem, 1)                # 1
            vector.wait_ge(dma_sem, 16)
            vector.tensor_scalar(
                out=g_sb[:], in0=a_sb[:], scalar1=0.0, scalar2=2.0,
                op0=mybir.AluOpType.is_ge, op1=mybir.AluOpType.mult,
            ).then_inc(sem, 1)                                             # 2
            vector.wait_ge(sem, 2)
            with nc.allow_low_precision(reason="counts <= 8, exact in bf16"):
                vector.tensor_reduce(
                    out=r1_sb[:],
                    in_=g_sb[:].rearrange("p (k o) -> p o k", k=KPP),
                    axis=mybir.AxisListType.X, op=mybir.AluOpType.add,
                ).then_inc(sem, 1)                                         # 3
            vector.wait_ge(sem, 4)  # matmul done
            vector.tensor_reduce(
                out=nb_sb[:], in_=c_ps[:],
                axis=mybir.AxisListType.X, op=mybir.AluOpType.max,
                negate=True,
            ).then_inc(sem, 1)                                             # 5
            vector.wait_ge(sem, 6)  # exp done
            vector.tensor_reduce(
                out=s_sb[:], in_=e_sb[:],
                axis=mybir.AxisListType.X, op=mybir.AluOpType.add,
            ).then_inc(sem, 1)                                             # 7
            vector.wait_ge(sem, 7)
            vector.reciprocal(out=r_sb[:], in_=s_sb[:]).then_inc(sem, 1)   # 8
            vector.wait_ge(sem, 8)
            for k in range(KPP):
                vector.tensor_scalar(
                    out=p_sb[:, k * O:(k + 1) * O], in0=e_sb[:],
                    scalar1=r_sb[:, 0:1], scalar2=None,
                    op0=mybir.AluOpType.mult,
                ).then_inc(sem, 1)                                         # 9..12

        @block.tensor
        def _(tensor):
            tensor.wait_ge(sem, 3)
            tensor.matmul(
                c_ps[:], ones_sb[:], r1_sb[:], start=True, stop=True,
            ).then_inc(sem, 1)                                             # 4

        @block.scalar
        def _(scalar):
            # Pre-warm the ACT exp table while the input DMA is in flight so
            # the real activation below skips the ~1.3us ACT_TABLE_LOAD.
            scalar.wait_ge(sem, 1)  # ones memset done
            scalar.activation(
                out=scr_sb[:], in_=ones_sb[0:1, 0:1],
                func=mybir.ActivationFunctionType.Exp,
            )
            scalar.wait_ge(sem, 5)
            scalar.activation(
                out=e_sb[:], in_=c_ps[:],
                func=mybir.ActivationFunctionType.Exp,
                bias=nb_sb[:, 0:1], scale=1.0,
            ).then_inc(sem, 1)                                             # 6

    return nc


def make_in_maps(params: dict) -> list[dict]:
    wo = np.ascontiguousarray(
        np.asarray(params["wo"], dtype=np.float32).reshape(P, FREE)
    )
    return [{"wo": wo} for _ in range(N_CORES)]


def assemble_out(results: list[dict]) -> np.ndarray:
    outs = [results[i]["out"].reshape(B_SHARD, T, O) for i in range(N_CORES)]
    return np.ascontiguousarray(np.concatenate(outs, axis=0), dtype=np.float32)


_NC_CACHE = None


def kernel(x: np.ndarray, params: dict) -> np.ndarray:
    global _NC_CACHE
    if _NC_CACHE is None:
        _NC_CACHE = build_nc()
    # x is dead in the reference function and is not shipped to the device.
    res = run_bass_kernel_spmd(_NC_CACHE, make_in_maps(params), list(range(N_CORES)))
    return assemble_out(res.results)
